# revision 1
# baseline (speedup 1.0000x reference)
"""GatedAttentionUnit Trainium2 kernel.

Strategy (8 NeuronCores, two SPMD launches):
  Launch 1 (data-parallel projections, 1024 rows/core):
    v = silu(x@Wv+bv)            -> natural layout [row, i]
    gateT = silu(x@Wg+bg)^T      -> [i, row]
    x_ = silu(x@Wi+bi)           -> [h, row];  qT = x_*(gq/sqrt(I))+bq/sqrt(I),
                                               kT = x_*gk+bk
  Launch 2 (sequence-parallel attention, balanced causal pairing):
    core (b, j) handles query chunks j and 7-j (512 rows each) of batch b.
    Uniform SPMD: part A = 4 kv-block steps, part B = 8 steps; unused steps
    are zero-key padded (relu(0)^2 = 0 contributes nothing).
    a = relu(qT.T kT masked)^2 ; oT = v.T a ; ogT = oT * gateT ; y = og@Wo+bo

All matmuls use bf16 operands with fp32 PSUM accumulation.
"""
import os
import sys

for _p in ("/opt/trn_rl_repo", "/root/.axon_site/_ro/trn_rl_repo"):
    if os.path.isdir(_p) and _p not in sys.path:
        sys.path.insert(0, _p)

import numpy as np
import ml_dtypes

import concourse.bass as bass
import concourse.tile as tile
from concourse import bacc, mybir
from concourse.bass_utils import run_bass_kernel_spmd

BF16 = ml_dtypes.bfloat16
FP32 = np.float32
dt = mybir.dt

B, N, E, H, I = 2, 4096, 1024, 128, 2048
NC = 8
CH = 512            # query chunk / kv block size
RPC = 2 * CH        # rows per core
ET = E // 128       # 8 contraction tiles
IT = I // 128       # 16 i tiles
LA, LB = 4, 8       # padded kv-step counts for parts A and B
NSTEPS = LA + LB
SCALE = float(I) ** 0.5

Silu = mybir.ActivationFunctionType.Silu
Relu = mybir.ActivationFunctionType.Relu
Copy = mybir.ActivationFunctionType.Copy
MULT = mybir.AluOpType.mult
ADD = mybir.AluOpType.add

_PROG_CACHE = {}


def _build_l1(has_bv, has_bg):
    nc = bacc.Bacc("TRN2", target_bir_lowering=False, debug=False, num_devices=NC)
    xT = nc.dram_tensor("xT", [E, RPC], dt.bfloat16, kind="ExternalInput").ap()
    Wv = nc.dram_tensor("Wv", [E, I], dt.bfloat16, kind="ExternalInput").ap()
    Wg = nc.dram_tensor("Wg", [E, I], dt.bfloat16, kind="ExternalInput").ap()
    Wi = nc.dram_tensor("Wi", [E, H], dt.bfloat16, kind="ExternalInput").ap()
    gq = nc.dram_tensor("gq", [H, 1], dt.float32, kind="ExternalInput").ap()
    bq = nc.dram_tensor("bq", [H, 1], dt.float32, kind="ExternalInput").ap()
    gk = nc.dram_tensor("gk", [H, 1], dt.float32, kind="ExternalInput").ap()
    bk = nc.dram_tensor("bk", [H, 1], dt.float32, kind="ExternalInput").ap()
    bi = nc.dram_tensor("bi", [H, 1], dt.float32, kind="ExternalInput").ap()
    bvr = nc.dram_tensor("bvr", [1, I], dt.bfloat16, kind="ExternalInput").ap()
    bgr = nc.dram_tensor("bgr", [1, I], dt.bfloat16, kind="ExternalInput").ap()
    v_out = nc.dram_tensor("v_out", [RPC, I], dt.bfloat16, kind="ExternalOutput").ap()
    gT_out = nc.dram_tensor("gT_out", [I, RPC], dt.bfloat16, kind="ExternalOutput").ap()
    qT_out = nc.dram_tensor("qT_out", [H, RPC], dt.bfloat16, kind="ExternalOutput").ap()
    kT_out = nc.dram_tensor("kT_out", [H, RPC], dt.bfloat16, kind="ExternalOutput").ap()

    with tile.TileContext(nc) as tc:
        with (
            tc.tile_pool(name="wts", bufs=1) as wts,
            tc.tile_pool(name="io", bufs=3) as io,
            tc.tile_pool(name="ps", bufs=2, space="PSUM") as ps,
        ):
            # PE warmup spin: independent matmuls that run during the DMA
            # head so the HAM clock-gate opens before real work arrives.
            warm_sb = wts.tile([128, CH], dt.bfloat16, tag="warm", name="warm_sb")
            nc.gpsimd.memset(warm_sb[:], 0.0)
            warm_ps = ps.tile([128, CH], dt.float32, tag="vps", name="warm_ps")
            for w in range(16):
                nc.tensor.matmul(warm_ps[:], warm_sb[:, 0:128], warm_sb[:],
                                 start=True, stop=True)

            # load order: xT + Wi first (unblocks xq then v), Wv next, Wg last
            xT_sb = []
            Wv_sb = []
            Wg_sb = []
            Wi_sb = []
            for e in range(ET):
                xt = wts.tile([128, RPC], dt.bfloat16, tag=f"xT{e}", name=f"xT{e}")
                nc.sync.dma_start(xt[:], xT[e * 128:(e + 1) * 128, :])
                xT_sb.append(xt)
                wi = wts.tile([128, H], dt.bfloat16, tag=f"Wi{e}", name=f"Wi{e}")
                nc.sync.dma_start(wi[:], Wi[e * 128:(e + 1) * 128, :])
                Wi_sb.append(wi)
            Wv_sb = {}
            for ib in range(I // CH):
                for e in range(ET):
                    wv = wts.tile([128, CH], dt.bfloat16, tag=f"Wv{e}_{ib}",
                                  name=f"Wv{e}_{ib}")
                    nc.sync.dma_start(
                        wv[:], Wv[e * 128:(e + 1) * 128, ib * CH:(ib + 1) * CH])
                    Wv_sb[(e, ib)] = wv
            for e in range(ET):
                Wg_sb.append(wts.tile([128, I], dt.bfloat16, tag=f"Wg{e}",
                                      name=f"Wg{e}"))
            gq_sb = wts.tile([H, 1], dt.float32, tag="gq", name="gq_sb")
            nc.sync.dma_start(gq_sb[:], gq[:])
            bq_sb = wts.tile([H, 1], dt.float32, tag="bq", name="bq_sb")
            nc.sync.dma_start(bq_sb[:], bq[:])
            gk_sb = wts.tile([H, 1], dt.float32, tag="gk", name="gk_sb")
            nc.sync.dma_start(gk_sb[:], gk[:])
            bk_sb = wts.tile([H, 1], dt.float32, tag="bk", name="bk_sb")
            nc.sync.dma_start(bk_sb[:], bk[:])
            bi_sb = wts.tile([H, 1], dt.float32, tag="bi", name="bi_sb")
            nc.sync.dma_start(bi_sb[:], bi[:])
            if has_bv or has_bg:
                ones_sb = wts.tile([1, CH], dt.bfloat16, tag="ones", name="ones_sb")
                nc.gpsimd.memset(ones_sb[:], 1.0)
            if has_bv:
                bvr_sb = wts.tile([1, I], dt.bfloat16, tag="bvr", name="bvr_sb")
                nc.sync.dma_start(bvr_sb[:], bvr[:])
            if has_bg:
                bgr_sb = wts.tile([1, I], dt.bfloat16, tag="bgr", name="bgr_sb")
                nc.sync.dma_start(bgr_sb[:], bgr[:])

            for e in range(ET):
                nc.sync.dma_start(Wg_sb[e][:], Wg[e * 128:(e + 1) * 128, :])

            # x_ -> qT, kT   [h, row]
            for rb in range(RPC // CH):
                xq_ps = ps.tile([128, CH], dt.float32, tag="xqps", name="xq_ps")
                for e in range(ET):
                    nc.tensor.matmul(
                        xq_ps[:], Wi_sb[e][:, 0:H],
                        xT_sb[e][:, rb * CH:(rb + 1) * CH],
                        start=(e == 0), stop=(e == ET - 1),
                    )
                xq_sb = io.tile([128, CH], dt.float32, tag="xq", name="xq_sb")
                nc.scalar.activation(xq_sb[:], xq_ps[:], Silu, bias=bi_sb[:])
                qT_sb = io.tile([128, CH], dt.bfloat16, tag="qt", name="qT_sb")
                nc.vector.tensor_scalar(qT_sb[:], xq_sb[:], gq_sb[:], bq_sb[:], MULT, ADD)
                nc.sync.dma_start(qT_out[:, rb * CH:(rb + 1) * CH], qT_sb[:])
                kT_sb = io.tile([128, CH], dt.bfloat16, tag="kt", name="kT_sb")
                nc.vector.tensor_scalar(kT_sb[:], xq_sb[:], gk_sb[:], bk_sb[:], MULT, ADD)
                nc.sync.dma_start(kT_out[:, rb * CH:(rb + 1) * CH], kT_sb[:])

            # v  [row, i]  (ib outer so Wv column tiles pipeline under compute)
            for ib in range(I // CH):
                for rt in range(RPC // 128):
                    v_ps = ps.tile([128, CH], dt.float32, tag="vps", name="v_ps")
                    for e in range(ET):
                        nc.tensor.matmul(
                            v_ps[:], xT_sb[e][:, rt * 128:(rt + 1) * 128],
                            Wv_sb[(e, ib)][:],
                            start=(e == 0), stop=(e == ET - 1 and not has_bv),
                        )
                    if has_bv:
                        nc.tensor.matmul(
                            v_ps[:], ones_sb[:, 0:128],
                            bvr_sb[:, ib * CH:(ib + 1) * CH],
                            start=False, stop=True,
                        )
                    v_sb = io.tile([128, CH], dt.bfloat16, tag="vsb", bufs=12, name="v_sb")
                    nc.scalar.activation(v_sb[:], v_ps[:], Silu)
                    nc.sync.dma_start(
                        v_out[rt * 128:(rt + 1) * 128, ib * CH:(ib + 1) * CH], v_sb[:])

            # gateT  [i, row]
            for it in range(IT):
                for rb in range(RPC // CH):
                    g_ps = ps.tile([128, CH], dt.float32, tag="gps", name="g_ps")
                    for e in range(ET):
                        nc.tensor.matmul(
                            g_ps[:], Wg_sb[e][:, it * 128:(it + 1) * 128],
                            xT_sb[e][:, rb * CH:(rb + 1) * CH],
                            start=(e == 0), stop=(e == ET - 1 and not has_bg),
                        )
                    if has_bg:
                        nc.tensor.matmul(
                            g_ps[:], bgr_sb[:, it * 128:(it + 1) * 128],
                            ones_sb[:, 0:CH],
                            start=False, stop=True,
                        )
                    g_sb = io.tile([128, CH], dt.bfloat16, tag="gsb", bufs=8, name="g_sb")
                    nc.scalar.activation(g_sb[:], g_ps[:], Silu)
                    nc.sync.dma_start(
                        gT_out[it * 128:(it + 1) * 128, rb * CH:(rb + 1) * CH], g_sb[:])

    nc.compile()
    return nc


def _build_l2(has_bo):
    nc = bacc.Bacc("TRN2", target_bir_lowering=False, debug=False, num_devices=NC)
    qT = nc.dram_tensor("qT", [H, RPC], dt.bfloat16, kind="ExternalInput").ap()
    gT = nc.dram_tensor("gT", [I, RPC], dt.bfloat16, kind="ExternalInput").ap()
    Wo = nc.dram_tensor("Wo", [I, E], dt.bfloat16, kind="ExternalInput").ap()
    kv_k = nc.dram_tensor("kv_k", [NSTEPS, H, CH], dt.bfloat16, kind="ExternalInput").ap()
    kv_v = nc.dram_tensor("kv_v", [NSTEPS, 2, 128, 2, I], dt.bfloat16, kind="ExternalInput").ap()
    Mext = nc.dram_tensor("Mext", [128, 896], dt.float32, kind="ExternalInput").ap()
    bor = nc.dram_tensor("bor", [1, E], dt.bfloat16, kind="ExternalInput").ap()
    y = nc.dram_tensor("y", [RPC, E], dt.float32, kind="ExternalOutput").ap()

    with tile.TileContext(nc) as tc:
        with (
            tc.tile_pool(name="wts", bufs=1) as wts,
            tc.tile_pool(name="kv", bufs=8) as kv,
            tc.tile_pool(name="ap", bufs=1) as apool,
            tc.tile_pool(name="tp", bufs=3) as tp,
            tc.tile_pool(name="og", bufs=1) as ogp,
            tc.tile_pool(name="ys", bufs=3) as ysp,
            tc.tile_pool(name="sp", bufs=2, space="PSUM") as sp,
            tc.tile_pool(name="op", bufs=1, space="PSUM") as op,
            tc.tile_pool(name="yp", bufs=2, space="PSUM") as yp,
        ):
            # PE warmup spin (overlaps the resident-load DMA head)
            warm_sb = wts.tile([128, CH], dt.bfloat16, tag="warm", name="warm_sb")
            nc.gpsimd.memset(warm_sb[:], 0.0)
            warm_ps = sp.tile([128, CH], dt.float32, tag="sps", name="warm_ps")
            for w in range(16):
                nc.tensor.matmul(warm_ps[:], warm_sb[:, 0:128], warm_sb[:],
                                 start=True, stop=True)

            qT_sb = wts.tile([H, RPC], dt.bfloat16, tag="qT", name="qT_sb")
            nc.sync.dma_start(qT_sb[:], qT[:])
            Mext_sb = wts.tile([128, 896], dt.float32, tag="Mext", name="Mext_sb")
            nc.sync.dma_start(Mext_sb[:], Mext[:])
            # gT/Wo tiles are allocated here but DMA'd after the part-A
            # scores are emitted, so kt/qT loads win the early HBM bandwidth.
            gT_sb = []
            Wo_sb = []
            for it in range(IT):
                gT_sb.append(wts.tile([128, RPC], dt.bfloat16, tag=f"gT{it}",
                                      name=f"gT{it}"))
                Wo_sb.append(wts.tile([128, E], dt.bfloat16, tag=f"Wo{it}",
                                      name=f"Wo{it}"))
            if has_bo:
                ones_sb = wts.tile([1, 128], dt.bfloat16, tag="ones", name="ones_sb")
                nc.gpsimd.memset(ones_sb[:], 1.0)
                bor_sb = wts.tile([1, E], dt.bfloat16, tag="bor", name="bor_sb")
                nc.sync.dma_start(bor_sb[:], bor[:])

            for part, (L, sbase) in enumerate(((LA, 0), (LB, LA))):
                rcol = slice(part * CH, (part + 1) * CH)
                # ---- scores -> a = relu(masked s)^2 (bf16) ----
                a_sb = {}
                for t in range(L):
                    kt = kv.tile([H, CH], dt.bfloat16, tag="kt", name=f"kt{part}_{t}")
                    nc.sync.dma_start(kt[:], kv_k[sbase + t])
                    for mt in range(4):
                        s_ps = sp.tile([128, CH], dt.float32, tag="sps", name="s_ps")
                        nc.tensor.matmul(
                            s_ps[:], kt[:, mt * 128:(mt + 1) * 128], qT_sb[:, rcol],
                            start=True, stop=True,
                        )
                        if t == 0:  # diagonal block: causal mask
                            off = 384 - 128 * mt
                            nc.vector.tensor_mul(
                                s_ps[:], s_ps[:], Mext_sb[:, off:off + CH])
                        t_sb = tp.tile([128, CH], dt.float32, tag="tsb", name="t_sb")
                        nc.scalar.activation(t_sb[:], s_ps[:], Relu)
                        a = apool.tile([128, CH], dt.bfloat16, bufs=2,
                                       tag=f"a{t}_{mt}", name=f"a{t}_{mt}")
                        nc.vector.tensor_mul(a[:], t_sb[:], t_sb[:])
                        a_sb[(t, mt)] = a
                # ---- oT accumulation + gate ----
                og_sb = [None] * IT
                for iq in range(4):
                    if part == 0:
                        for j4 in range(4):
                            it4 = iq * 4 + j4
                            nc.sync.dma_start(
                                gT_sb[it4][:], gT[it4 * 128:(it4 + 1) * 128, :])
                            nc.sync.dma_start(
                                Wo_sb[it4][:], Wo[it4 * 128:(it4 + 1) * 128, :])
                    o_ps = [
                        op.tile([128, CH], dt.float32, tag=f"o{j}", name=f"o{j}")
                        for j in range(4)
                    ]
                    for t in range(L):
                        for mtp in range(2):
                            vt = kv.tile([128, 2 * CH], dt.bfloat16, tag="vt",
                                         name=f"vt{part}_{iq}_{t}_{mtp}")
                            nc.sync.dma_start(
                                vt[:],
                                kv_v[sbase + t, mtp, :, :,
                                     iq * CH:(iq + 1) * CH])
                            for sub in range(2):
                                mt = 2 * mtp + sub
                                for j in range(4):
                                    nc.tensor.matmul(
                                        o_ps[j][:],
                                        vt[:, sub * CH + j * 128:
                                           sub * CH + (j + 1) * 128],
                                        a_sb[(t, mt)][:],
                                        start=(t == 0 and mt == 0),
                                        stop=(t == L - 1 and mt == 3),
                                    )
                    for j in range(4):
                        it = iq * 4 + j
                        og = ogp.tile([128, CH], dt.bfloat16,
                                      tag=f"og{it}", name=f"og{it}")
                        nc.vector.tensor_mul(og[:], o_ps[j][:], gT_sb[it][:, rcol])
                        og_sb[it] = og
                # ---- y = og @ Wo (+ bo) ----
                for rt in range(4):
                    for eb in range(E // CH):
                        y_ps = yp.tile([128, CH], dt.float32, tag="yps", name="y_ps")
                        for it in range(IT):
                            nc.tensor.matmul(
                                y_ps[:], og_sb[it][:, rt * 128:(rt + 1) * 128],
                                Wo_sb[it][:, eb * CH:(eb + 1) * CH],
                                start=(it == 0), stop=(it == IT - 1 and not has_bo),
                            )
                        if has_bo:
                            nc.tensor.matmul(
                                y_ps[:], ones_sb[:, 0:128],
                                bor_sb[:, eb * CH:(eb + 1) * CH],
                                start=False, stop=True,
                            )
                        y_sb = ysp.tile([128, CH], dt.float32, tag="ysb", bufs=6, name="y_sb")
                        nc.scalar.activation(y_sb[:], y_ps[:], Copy)
                        nc.sync.dma_start(
                            y[part * CH + rt * 128:part * CH + (rt + 1) * 128,
                              eb * CH:(eb + 1) * CH], y_sb[:])

    nc.compile()
    return nc


def _get_prog(which, *flags):
    key = (which,) + flags
    if key not in _PROG_CACHE:
        _PROG_CACHE[key] = _build_l1(*flags) if which == 1 else _build_l2(*flags)
    return _PROG_CACHE[key]


# core -> (batch, chunkA, chunkB): balanced causal pairing
_ASSIGN = [(b, j, 7 - j) for b in range(B) for j in range(4)]


def kernel(x, Wv, bv, Wg, bg, Wi, bi, gamma_q, beta_q, gamma_k, beta_k, Wo, bo):
    x = np.asarray(x, FP32)
    Wv = np.asarray(Wv, FP32); bv = np.asarray(bv, FP32)
    Wg = np.asarray(Wg, FP32); bg = np.asarray(bg, FP32)
    Wi = np.asarray(Wi, FP32); bi = np.asarray(bi, FP32)
    gamma_q = np.asarray(gamma_q, FP32); beta_q = np.asarray(beta_q, FP32)
    gamma_k = np.asarray(gamma_k, FP32); beta_k = np.asarray(beta_k, FP32)
    Wo = np.asarray(Wo, FP32); bo = np.asarray(bo, FP32)

    has_bv = bool(np.any(bv)); has_bg = bool(np.any(bg)); has_bo = bool(np.any(bo))
    nc1 = _get_prog(1, has_bv, has_bg)
    nc2 = _get_prog(2, has_bo)

    Wv_b = Wv.astype(BF16); Wg_b = Wg.astype(BF16); Wi_b = Wi.astype(BF16)
    Wo_b = Wo.astype(BF16)
    gq = (gamma_q / SCALE).astype(FP32).reshape(H, 1)
    bq = (beta_q / SCALE).astype(FP32).reshape(H, 1)
    gk = gamma_k.reshape(H, 1).copy(); bk = beta_k.reshape(H, 1).copy()
    bi_c = bi.reshape(H, 1).copy()
    bvr = bv.reshape(1, I).astype(BF16); bgr = bg.reshape(1, I).astype(BF16)
    bor = bo.reshape(1, E).astype(BF16)

    xg = x.reshape(B, N // CH, CH, E)
    in1 = []
    for (b, cA, cB) in _ASSIGN:
        xc = np.concatenate([xg[b, cA], xg[b, cB]], axis=0)  # [RPC, E]
        in1.append({
            "xT": np.ascontiguousarray(xc.T).astype(BF16),
            "Wv": Wv_b, "Wg": Wg_b, "Wi": Wi_b,
            "gq": gq, "bq": bq, "gk": gk, "bk": bk, "bi": bi_c,
            "bvr": bvr, "bgr": bgr,
        })
    res1 = run_bass_kernel_spmd(nc1, in1, core_ids=list(range(NC)))

    vfull = np.zeros((B, N // CH, CH, I), BF16)
    kTfull = np.zeros((B, H, N), BF16)
    for c, (b, cA, cB) in enumerate(_ASSIGN):
        r = res1.results[c]
        vfull[b, cA] = r["v_out"][:CH]
        vfull[b, cB] = r["v_out"][CH:]
        kTfull[b][:, cA * CH:(cA + 1) * CH] = r["kT_out"][:, :CH]
        kTfull[b][:, cB * CH:(cB + 1) * CH] = r["kT_out"][:, CH:]

    # extended causal mask: Mext[p, u] = 1 iff u >= p + 384
    Mext = (np.arange(896)[None, :] >= (np.arange(128)[:, None] + 384)).astype(FP32)

    in2 = []
    for c, (b, cA, cB) in enumerate(_ASSIGN):
        r = res1.results[c]
        kvk = np.zeros((NSTEPS, H, CH), BF16)
        kvv = np.zeros((NSTEPS, CH, I), BF16)
        for base, cq in ((0, cA), (LA, cB)):
            kvk[base] = kTfull[b][:, cq * CH:(cq + 1) * CH]
            kvv[base] = vfull[b, cq]
            for idx in range(cq):
                kvk[base + 1 + idx] = kTfull[b][:, idx * CH:(idx + 1) * CH]
                kvv[base + 1 + idx] = vfull[b, idx]
        kvv2 = np.ascontiguousarray(
            kvv.reshape(NSTEPS, 2, 2, 128, I).transpose(0, 1, 3, 2, 4))
        in2.append({
            "qT": r["qT_out"], "gT": r["gT_out"], "Wo": Wo_b,
            "kv_k": kvk, "kv_v": kvv2, "Mext": Mext, "bor": bor,
        })
    res2 = run_bass_kernel_spmd(nc2, in2, core_ids=list(range(NC)))

    out = np.zeros((B, N // CH, CH, E), FP32)
    for c, (b, cA, cB) in enumerate(_ASSIGN):
        yy = res2.results[c]["y"]
        out[b, cA] = yy[:CH]
        out[b, cB] = yy[CH:]
    return out.reshape(B, N, E)



# revision 9
# speedup vs baseline: 1.4536x; 1.4536x over previous
"""GatedAttentionUnit Trainium2 kernel.

Strategy (8 NeuronCores, two SPMD launches):
  Launch 1 (data-parallel projections, 1024 rows/core):
    v = silu(x@Wv+bv)            -> natural layout [row, i]
    gateT = silu(x@Wg+bg)^T      -> [i, row]
    x_ = silu(x@Wi+bi)           -> [h, row];  qT = x_*(16*gq/sqrt(I))+16*bq/sqrt(I),
                                               kT = x_*gk+bk
  Launch 2 (sequence-parallel attention, balanced causal pairing):
    core (b, j) handles query chunks j and 7-j (512 rows each) of batch b.
    Uniform SPMD: part A = 4 kv-block steps, part B = 8 steps; unused steps
    are zero-key padded (relu(0)^2 = 0 contributes nothing).
    a = relu(s)*s with s pre-scaled x16  -> a = 256*relu^2, cast to fp8 e4m3
    oT = v8.T a via fp8 DoubleRow matmuls (v8 = 64*v in e4m3, K=256/instr)
    ogT = oT * gateT/2^14 (undoes 256*64) ; y = og@Wo+bo  (bf16)

Projections/scores/output matmuls are bf16; the dominant attention*V
matmul runs in fp8 DoubleRow. Scale factors keep a/v in e4m3's normal
range (a mean ~2e-3 would otherwise hit denormals) and cost zero extra
on-chip ops: x16 folds into gamma_q, x64 into the host v->fp8 cast, and
the 2^-14 correction into the host-prepared gate.
"""
import os
import sys

for _p in ("/opt/trn_rl_repo", "/root/.axon_site/_ro/trn_rl_repo"):
    if os.path.isdir(_p) and _p not in sys.path:
        sys.path.insert(0, _p)

import numpy as np
import ml_dtypes

import concourse.bass as bass
import concourse.tile as tile
from concourse import bacc, mybir
from concourse.bass_utils import run_bass_kernel_spmd

BF16 = ml_dtypes.bfloat16
FP32 = np.float32
dt = mybir.dt

B, N, E, H, I = 2, 4096, 1024, 128, 2048
NC = 8
CH = 512            # query chunk / kv block size
RPC = 2 * CH        # rows per core
ET = E // 128       # 8 contraction tiles
IT = I // 128       # 16 i tiles
LA, LB = 4, 8       # padded kv-step counts for parts A and B
NSTEPS = LA + LB
SCALE = float(I) ** 0.5

Silu = mybir.ActivationFunctionType.Silu
Relu = mybir.ActivationFunctionType.Relu
Copy = mybir.ActivationFunctionType.Copy
MULT = mybir.AluOpType.mult
ADD = mybir.AluOpType.add
MAX = mybir.AluOpType.max
DR = mybir.MatmulPerfMode.DoubleRow
E4M3 = ml_dtypes.float8_e4m3fn
S_SCALE = 16.0       # scores pre-scale (via gamma_q); a = relu(s)^2 gets x256
V_SCALE = 64.0       # v -> fp8 pre-scale
OG_SCALE = S_SCALE * S_SCALE * V_SCALE  # gate is divided by this (2^14)

_PROG_CACHE = {}


def _build_l1(has_bv, has_bg):
    nc = bacc.Bacc("TRN2", target_bir_lowering=False, debug=False, num_devices=NC)
    xT = nc.dram_tensor("xT", [E, RPC], dt.bfloat16, kind="ExternalInput").ap()
    Wv = nc.dram_tensor("Wv", [E, I], dt.bfloat16, kind="ExternalInput").ap()
    Wg = nc.dram_tensor("Wg", [E, I], dt.bfloat16, kind="ExternalInput").ap()
    Wi = nc.dram_tensor("Wi", [E, H], dt.bfloat16, kind="ExternalInput").ap()
    gq = nc.dram_tensor("gq", [H, 1], dt.float32, kind="ExternalInput").ap()
    bq = nc.dram_tensor("bq", [H, 1], dt.float32, kind="ExternalInput").ap()
    gk = nc.dram_tensor("gk", [H, 1], dt.float32, kind="ExternalInput").ap()
    bk = nc.dram_tensor("bk", [H, 1], dt.float32, kind="ExternalInput").ap()
    bi = nc.dram_tensor("bi", [H, 1], dt.float32, kind="ExternalInput").ap()
    bvr = nc.dram_tensor("bvr", [1, I], dt.bfloat16, kind="ExternalInput").ap()
    bgr = nc.dram_tensor("bgr", [1, I], dt.bfloat16, kind="ExternalInput").ap()
    v_out = nc.dram_tensor("v_out", [RPC, I], dt.bfloat16, kind="ExternalOutput").ap()
    gT_out = nc.dram_tensor("gT_out", [I, RPC], dt.bfloat16, kind="ExternalOutput").ap()
    qT_out = nc.dram_tensor("qT_out", [H, RPC], dt.bfloat16, kind="ExternalOutput").ap()
    kT_out = nc.dram_tensor("kT_out", [H, RPC], dt.bfloat16, kind="ExternalOutput").ap()

    with tile.TileContext(nc) as tc:
        with (
            tc.tile_pool(name="wts", bufs=1) as wts,
            tc.tile_pool(name="io", bufs=3) as io,
            tc.tile_pool(name="ps", bufs=2, space="PSUM") as ps,
        ):
            # PE warmup spin: independent matmuls that run during the DMA
            # head so the HAM clock-gate opens before real work arrives.
            warm_sb = wts.tile([128, CH], dt.bfloat16, tag="warm", name="warm_sb")
            nc.gpsimd.memset(warm_sb[:], 0.0)
            warm_ps = ps.tile([128, CH], dt.float32, tag="vps", name="warm_ps")
            for w in range(16):
                nc.tensor.matmul(warm_ps[:], warm_sb[:, 0:128], warm_sb[:],
                                 start=True, stop=True)

            # load order: xT + Wi first (unblocks xq then v), Wv next, Wg last
            xT_sb = []
            Wv_sb = []
            Wg_sb = []
            Wi_sb = []
            for e in range(ET):
                xt = wts.tile([128, RPC], dt.bfloat16, tag=f"xT{e}", name=f"xT{e}")
                nc.sync.dma_start(xt[:], xT[e * 128:(e + 1) * 128, :])
                xT_sb.append(xt)
                wi = wts.tile([128, H], dt.bfloat16, tag=f"Wi{e}", name=f"Wi{e}")
                nc.sync.dma_start(wi[:], Wi[e * 128:(e + 1) * 128, :])
                Wi_sb.append(wi)
            Wv_sb = {}
            for ib in range(I // CH):
                for e in range(ET):
                    wv = wts.tile([128, CH], dt.bfloat16, tag=f"Wv{e}_{ib}",
                                  name=f"Wv{e}_{ib}")
                    nc.sync.dma_start(
                        wv[:], Wv[e * 128:(e + 1) * 128, ib * CH:(ib + 1) * CH])
                    Wv_sb[(e, ib)] = wv
            for e in range(ET):
                Wg_sb.append(wts.tile([128, I], dt.bfloat16, tag=f"Wg{e}",
                                      name=f"Wg{e}"))
            gq_sb = wts.tile([H, 1], dt.float32, tag="gq", name="gq_sb")
            nc.sync.dma_start(gq_sb[:], gq[:])
            bq_sb = wts.tile([H, 1], dt.float32, tag="bq", name="bq_sb")
            nc.sync.dma_start(bq_sb[:], bq[:])
            gk_sb = wts.tile([H, 1], dt.float32, tag="gk", name="gk_sb")
            nc.sync.dma_start(gk_sb[:], gk[:])
            bk_sb = wts.tile([H, 1], dt.float32, tag="bk", name="bk_sb")
            nc.sync.dma_start(bk_sb[:], bk[:])
            bi_sb = wts.tile([H, 1], dt.float32, tag="bi", name="bi_sb")
            nc.sync.dma_start(bi_sb[:], bi[:])
            if has_bv or has_bg:
                ones_sb = wts.tile([1, CH], dt.bfloat16, tag="ones", name="ones_sb")
                nc.gpsimd.memset(ones_sb[:], 1.0)
            if has_bv:
                bvr_sb = wts.tile([1, I], dt.bfloat16, tag="bvr", name="bvr_sb")
                nc.sync.dma_start(bvr_sb[:], bvr[:])
            if has_bg:
                bgr_sb = wts.tile([1, I], dt.bfloat16, tag="bgr", name="bgr_sb")
                nc.sync.dma_start(bgr_sb[:], bgr[:])

            for e in range(ET):
                nc.sync.dma_start(Wg_sb[e][:], Wg[e * 128:(e + 1) * 128, :])

            # x_ -> qT, kT   [h, row]
            for rb in range(RPC // CH):
                xq_ps = ps.tile([128, CH], dt.float32, tag="xqps", name="xq_ps")
                for e in range(ET):
                    nc.tensor.matmul(
                        xq_ps[:], Wi_sb[e][:, 0:H],
                        xT_sb[e][:, rb * CH:(rb + 1) * CH],
                        start=(e == 0), stop=(e == ET - 1),
                    )
                xq_sb = io.tile([128, CH], dt.float32, tag="xq", name="xq_sb")
                nc.scalar.activation(xq_sb[:], xq_ps[:], Silu, bias=bi_sb[:])
                qT_sb = io.tile([128, CH], dt.bfloat16, tag="qt", name="qT_sb")
                nc.vector.tensor_scalar(qT_sb[:], xq_sb[:], gq_sb[:], bq_sb[:], MULT, ADD)
                nc.sync.dma_start(qT_out[:, rb * CH:(rb + 1) * CH], qT_sb[:])
                kT_sb = io.tile([128, CH], dt.bfloat16, tag="kt", name="kT_sb")
                nc.vector.tensor_scalar(kT_sb[:], xq_sb[:], gk_sb[:], bk_sb[:], MULT, ADD)
                nc.sync.dma_start(kT_out[:, rb * CH:(rb + 1) * CH], kT_sb[:])

            # v  [row, i]  (ib outer so Wv column tiles pipeline under compute)
            for ib in range(I // CH):
                for rt in range(RPC // 128):
                    v_ps = ps.tile([128, CH], dt.float32, tag="vps", name="v_ps")
                    for e in range(ET):
                        nc.tensor.matmul(
                            v_ps[:], xT_sb[e][:, rt * 128:(rt + 1) * 128],
                            Wv_sb[(e, ib)][:],
                            start=(e == 0), stop=(e == ET - 1 and not has_bv),
                        )
                    if has_bv:
                        nc.tensor.matmul(
                            v_ps[:], ones_sb[:, 0:128],
                            bvr_sb[:, ib * CH:(ib + 1) * CH],
                            start=False, stop=True,
                        )
                    v_sb = io.tile([128, CH], dt.bfloat16, tag="vsb", bufs=12, name="v_sb")
                    nc.scalar.activation(v_sb[:], v_ps[:], Silu)
                    nc.sync.dma_start(
                        v_out[rt * 128:(rt + 1) * 128, ib * CH:(ib + 1) * CH], v_sb[:])

            # gateT  [i, row]
            for it in range(IT):
                for rb in range(RPC // CH):
                    g_ps = ps.tile([128, CH], dt.float32, tag="gps", name="g_ps")
                    for e in range(ET):
                        nc.tensor.matmul(
                            g_ps[:], Wg_sb[e][:, it * 128:(it + 1) * 128],
                            xT_sb[e][:, rb * CH:(rb + 1) * CH],
                            start=(e == 0), stop=(e == ET - 1 and not has_bg),
                        )
                    if has_bg:
                        nc.tensor.matmul(
                            g_ps[:], bgr_sb[:, it * 128:(it + 1) * 128],
                            ones_sb[:, 0:CH],
                            start=False, stop=True,
                        )
                    g_sb = io.tile([128, CH], dt.bfloat16, tag="gsb", bufs=8, name="g_sb")
                    nc.scalar.activation(g_sb[:], g_ps[:], Silu)
                    nc.sync.dma_start(
                        gT_out[it * 128:(it + 1) * 128, rb * CH:(rb + 1) * CH], g_sb[:])

    nc.compile()
    return nc


def _build_l2(has_bo):
    nc = bacc.Bacc("TRN2", target_bir_lowering=False, debug=False, num_devices=NC)
    qT = nc.dram_tensor("qT", [H, RPC], dt.bfloat16, kind="ExternalInput").ap()
    gT = nc.dram_tensor("gT", [I, RPC], dt.bfloat16, kind="ExternalInput").ap()
    Wo = nc.dram_tensor("Wo", [I, E], dt.bfloat16, kind="ExternalInput").ap()
    kv_k = nc.dram_tensor("kv_k", [NSTEPS, H, CH], dt.bfloat16, kind="ExternalInput").ap()
    # [step, kv-pair(256), kv_lo(128), kv_half, i] fp8: DoubleRow layout
    kv_v = nc.dram_tensor("kv_v", [NSTEPS, 2, 128, 2, I], dt.float8e4, kind="ExternalInput").ap()
    Mext = nc.dram_tensor("Mext", [128, 896], dt.float32, kind="ExternalInput").ap()
    bor = nc.dram_tensor("bor", [1, E], dt.bfloat16, kind="ExternalInput").ap()
    y = nc.dram_tensor("y", [RPC, E], dt.float32, kind="ExternalOutput").ap()

    with tile.TileContext(nc) as tc:
        with (
            tc.tile_pool(name="wts", bufs=1) as wts,
            tc.tile_pool(name="kv", bufs=8) as kv,
            tc.tile_pool(name="ap", bufs=1) as apool,
            tc.tile_pool(name="tp", bufs=3) as tp,
            tc.tile_pool(name="og", bufs=1) as ogp,
            tc.tile_pool(name="ys", bufs=3) as ysp,
            tc.tile_pool(name="sp", bufs=2, space="PSUM") as sp,
            tc.tile_pool(name="op", bufs=1, space="PSUM") as op,
            tc.tile_pool(name="yp", bufs=2, space="PSUM") as yp,
        ):
            # PE warmup spin (overlaps the resident-load DMA head)
            warm_sb = wts.tile([128, CH], dt.bfloat16, tag="warm", name="warm_sb")
            nc.gpsimd.memset(warm_sb[:], 0.0)
            warm_ps = sp.tile([128, CH], dt.float32, tag="sps", name="warm_ps")
            for w in range(16):
                nc.tensor.matmul(warm_ps[:], warm_sb[:, 0:128], warm_sb[:],
                                 start=True, stop=True)

            qT_sb = wts.tile([H, RPC], dt.bfloat16, tag="qT", name="qT_sb")
            nc.sync.dma_start(qT_sb[:], qT[:])
            Mext_sb = wts.tile([128, 896], dt.float32, tag="Mext", name="Mext_sb")
            nc.sync.dma_start(Mext_sb[:], Mext[:])
            # gT/Wo tiles are allocated here but DMA'd after the part-A
            # scores are emitted, so kt/qT loads win the early HBM bandwidth.
            gT_sb = []
            Wo_sb = []
            for it in range(IT):
                gT_sb.append(wts.tile([128, RPC], dt.bfloat16, tag=f"gT{it}",
                                      name=f"gT{it}"))
                Wo_sb.append(wts.tile([128, E], dt.bfloat16, tag=f"Wo{it}",
                                      name=f"Wo{it}"))
            if has_bo:
                ones_sb = wts.tile([1, 128], dt.bfloat16, tag="ones", name="ones_sb")
                nc.gpsimd.memset(ones_sb[:], 1.0)
                bor_sb = wts.tile([1, E], dt.bfloat16, tag="bor", name="bor_sb")
                nc.sync.dma_start(bor_sb[:], bor[:])

            for part, (L, sbase) in enumerate(((LA, 0), (LB, LA))):
                rcol = slice(part * CH, (part + 1) * CH)
                # ---- scores -> a = relu(s)*s = 256*relu^2, cast fp8 ----
                # a_pair[(t,p)][:, h, :] holds kv block mt=2p+h
                a_sb = {}
                for t in range(L):
                    kt = kv.tile([H, CH], dt.bfloat16, tag="kt", name=f"kt{part}_{t}")
                    nc.sync.dma_start(kt[:], kv_k[sbase + t])
                    for p in range(2):
                        a = apool.tile([128, 2, CH], dt.float8e4, bufs=2,
                                       tag=f"a{t}_{p}", name=f"a{t}_{p}")
                        for h in range(2):
                            mt = 2 * p + h
                            s_ps = sp.tile([128, CH], dt.float32, tag="sps",
                                           name="s_ps")
                            nc.tensor.matmul(
                                s_ps[:], kt[:, mt * 128:(mt + 1) * 128],
                                qT_sb[:, rcol], start=True, stop=True,
                            )
                            if t == 0:  # diagonal block: causal mask
                                off = 384 - 128 * mt
                                nc.vector.tensor_mul(
                                    s_ps[:], s_ps[:], Mext_sb[:, off:off + CH])
                            t_sb = tp.tile([128, CH], dt.bfloat16, tag="tsb",
                                           name="t_sb")
                            nc.scalar.activation(t_sb[:], s_ps[:], Relu)
                            nc.vector.tensor_mul(a[:, h, :], t_sb[:], t_sb[:])
                        a_sb[(t, p)] = a
                # ---- oT accumulation (fp8 DoubleRow, K=256/instr) + gate ----
                og_sb = [None] * IT
                for iq in range(4):
                    if part == 0:
                        for j4 in range(4):
                            it4 = iq * 4 + j4
                            nc.sync.dma_start(
                                gT_sb[it4][:], gT[it4 * 128:(it4 + 1) * 128, :])
                            nc.sync.dma_start(
                                Wo_sb[it4][:], Wo[it4 * 128:(it4 + 1) * 128, :])
                    o_ps = [
                        op.tile([128, CH], dt.float32, tag=f"o{j}", name=f"o{j}")
                        for j in range(4)
                    ]
                    for t in range(L):
                        for p in range(2):
                            vt = kv.tile([128, 2, CH], dt.float8e4, tag="vt",
                                         name=f"vt{part}_{iq}_{t}_{p}")
                            nc.sync.dma_start(
                                vt[:],
                                kv_v[sbase + t, p, :, :, iq * CH:(iq + 1) * CH])
                            for j in range(4):
                                nc.tensor.matmul(
                                    o_ps[j][:],
                                    vt[:, :, j * 128:(j + 1) * 128],
                                    a_sb[(t, p)][:],
                                    start=(t == 0 and p == 0),
                                    stop=(t == L - 1 and p == 1),
                                    perf_mode=DR,
                                )
                    for j in range(4):
                        it = iq * 4 + j
                        og = ogp.tile([128, CH], dt.bfloat16,
                                      tag=f"og{it}", name=f"og{it}")
                        nc.vector.tensor_mul(og[:], o_ps[j][:], gT_sb[it][:, rcol])
                        og_sb[it] = og
                # ---- y = og @ Wo (+ bo) ----
                for rt in range(4):
                    for eb in range(E // CH):
                        y_ps = yp.tile([128, CH], dt.float32, tag="yps", name="y_ps")
                        for it in range(IT):
                            nc.tensor.matmul(
                                y_ps[:], og_sb[it][:, rt * 128:(rt + 1) * 128],
                                Wo_sb[it][:, eb * CH:(eb + 1) * CH],
                                start=(it == 0), stop=(it == IT - 1 and not has_bo),
                            )
                        if has_bo:
                            nc.tensor.matmul(
                                y_ps[:], ones_sb[:, 0:128],
                                bor_sb[:, eb * CH:(eb + 1) * CH],
                                start=False, stop=True,
                            )
                        y_sb = ysp.tile([128, CH], dt.float32, tag="ysb", bufs=6, name="y_sb")
                        nc.scalar.activation(y_sb[:], y_ps[:], Copy)
                        nc.sync.dma_start(
                            y[part * CH + rt * 128:part * CH + (rt + 1) * 128,
                              eb * CH:(eb + 1) * CH], y_sb[:])

    nc.compile()
    return nc


def _get_prog(which, *flags):
    key = (which,) + flags
    if key not in _PROG_CACHE:
        _PROG_CACHE[key] = _build_l1(*flags) if which == 1 else _build_l2(*flags)
    return _PROG_CACHE[key]


# core -> (batch, chunkA, chunkB): balanced causal pairing
_ASSIGN = [(b, j, 7 - j) for b in range(B) for j in range(4)]


def kernel(x, Wv, bv, Wg, bg, Wi, bi, gamma_q, beta_q, gamma_k, beta_k, Wo, bo):
    x = np.asarray(x, FP32)
    Wv = np.asarray(Wv, FP32); bv = np.asarray(bv, FP32)
    Wg = np.asarray(Wg, FP32); bg = np.asarray(bg, FP32)
    Wi = np.asarray(Wi, FP32); bi = np.asarray(bi, FP32)
    gamma_q = np.asarray(gamma_q, FP32); beta_q = np.asarray(beta_q, FP32)
    gamma_k = np.asarray(gamma_k, FP32); beta_k = np.asarray(beta_k, FP32)
    Wo = np.asarray(Wo, FP32); bo = np.asarray(bo, FP32)

    has_bv = bool(np.any(bv)); has_bg = bool(np.any(bg)); has_bo = bool(np.any(bo))
    nc1 = _get_prog(1, has_bv, has_bg)
    nc2 = _get_prog(2, has_bo)

    Wv_b = Wv.astype(BF16); Wg_b = Wg.astype(BF16); Wi_b = Wi.astype(BF16)
    Wo_b = Wo.astype(BF16)
    gq = (gamma_q * (S_SCALE / SCALE)).astype(FP32).reshape(H, 1)
    bq = (beta_q * (S_SCALE / SCALE)).astype(FP32).reshape(H, 1)
    gk = gamma_k.reshape(H, 1).copy(); bk = beta_k.reshape(H, 1).copy()
    bi_c = bi.reshape(H, 1).copy()
    bvr = bv.reshape(1, I).astype(BF16); bgr = bg.reshape(1, I).astype(BF16)
    bor = bo.reshape(1, E).astype(BF16)

    xg = x.reshape(B, N // CH, CH, E)
    in1 = []
    for (b, cA, cB) in _ASSIGN:
        xc = np.concatenate([xg[b, cA], xg[b, cB]], axis=0)  # [RPC, E]
        in1.append({
            "xT": np.ascontiguousarray(xc.T).astype(BF16),
            "Wv": Wv_b, "Wg": Wg_b, "Wi": Wi_b,
            "gq": gq, "bq": bq, "gk": gk, "bk": bk, "bi": bi_c,
            "bvr": bvr, "bgr": bgr,
        })
    res1 = run_bass_kernel_spmd(nc1, in1, core_ids=list(range(NC)))

    vfull = np.zeros((B, N // CH, CH, I), E4M3)
    kTfull = np.zeros((B, H, N), BF16)
    for c, (b, cA, cB) in enumerate(_ASSIGN):
        r = res1.results[c]
        v8 = (r["v_out"].astype(FP32) * V_SCALE).astype(E4M3)
        vfull[b, cA] = v8[:CH]
        vfull[b, cB] = v8[CH:]
        kTfull[b][:, cA * CH:(cA + 1) * CH] = r["kT_out"][:, :CH]
        kTfull[b][:, cB * CH:(cB + 1) * CH] = r["kT_out"][:, CH:]

    # extended causal mask: Mext[p, u] = 1 iff u >= p + 384
    Mext = (np.arange(896)[None, :] >= (np.arange(128)[:, None] + 384)).astype(FP32)

    in2 = []
    for c, (b, cA, cB) in enumerate(_ASSIGN):
        r = res1.results[c]
        kvk = np.zeros((NSTEPS, H, CH), BF16)
        kvv = np.zeros((NSTEPS, CH, I), E4M3)
        for base, cq in ((0, cA), (LA, cB)):
            kvk[base] = kTfull[b][:, cq * CH:(cq + 1) * CH]
            kvv[base] = vfull[b, cq]
            for idx in range(cq):
                kvk[base + 1 + idx] = kTfull[b][:, idx * CH:(idx + 1) * CH]
                kvv[base + 1 + idx] = vfull[b, idx]
        # [t, pair, kv_lo, kv_half, i] for DoubleRow lhsT tiles
        kvv2 = np.ascontiguousarray(
            kvv.reshape(NSTEPS, 2, 2, 128, I).transpose(0, 1, 3, 2, 4))
        gT_s = (r["gT_out"].astype(FP32) * (1.0 / OG_SCALE)).astype(BF16)
        in2.append({
            "qT": r["qT_out"], "gT": gT_s, "Wo": Wo_b,
            "kv_k": kvk, "kv_v": kvv2, "Mext": Mext, "bor": bor,
        })
    res2 = run_bass_kernel_spmd(nc2, in2, core_ids=list(range(NC)))

    out = np.zeros((B, N // CH, CH, E), FP32)
    for c, (b, cA, cB) in enumerate(_ASSIGN):
        yy = res2.results[c]["y"]
        out[b, cA] = yy[:CH]
        out[b, cB] = yy[CH:]
    return out.reshape(B, N, E)

